# revision 9
# baseline (speedup 1.0000x reference)
"""AntisymmetricRNN Trainium2 kernel — 8-core data-parallel over batch.

Math (per reference):
    mask = strictly-lower-tri; w_r = v_r * mask; A = w_r - w_r.T
    step:  h' = h + (1/TAU) * tanh( tanh(h) @ A + b_r - GAMMA*h )
           x_pred = tanh(h') @ w_o.T + b_o;   err_t = x_pred - x_t

Design (v3 "persistent bank", from the ~1586us baseline whose period was
bound by the bank-stop -> drain/sem -> ACT u -> DVE h+= -> ACT th ->
next-matmuls ring at ~3.1us/step):

  * device state is th (fp32 master `thm` + bf16 copy ring `TH`) and the
    PSUM bank itself.  h is never materialized.  Writing
    m_j = th_j - th_{j-1} = (1/TAU) * u_j * (1 - th_{j-1}^2)   (1st-order
    tanh update; measured end-to-end err ~4e-4 vs f64, budget 2e-2), the
    bank recurrence is
        z_{j+1} = z_j + A^T @ m_j - (GAMMA/TAU) * u_j
    so the bank is NEVER re-written: matmuls accumulate A^T m (64 MMs,
    rhs = m) plus 8 small diagonal matmuls (lhsT = -(G/TAU)*I, rhs = u)
    for the damping/bias delta.  No prewrite, no bank ping-pong.
  * the critical ring collapses to bank-stop -> (drain+sem ~490) ->
    ACT u=tanh(z) (bf16) -> DVE m = (u*invtau)*d (bf16, 2 el/cyc) ->
    next matmuls: ~1.15us, vs ~1.67us for the exact u->h->th chain.
  * th bookkeeping is off-ring with >= 1 period of slack:
    DVE: thb = thm + m (bf16 ring copy for the projection lhsT),
    GPSIMD: thm += m (fp32 master, ~0.4us/op measured),
    every 2nd step: ACT sq = thm^2, DVE d = 1 - sq  (d-lag validated).
  * 3 chunks {m0-2, m3-5, m6-7}; MM block order
    (0,0)(1,0)(0,1)(0,2)(1,1)(1,2)(2,0)(2,1)(2,2) staggers bank stops
    against consumption (event-sim: ~2.38us/step vs 3.26 for baseline).
  * output projection unchanged from baseline: one group (4 steps)
    behind, 2 k-bursts/step of 4 column-tiled matmuls into a [128,256]
    PSUM accumulator; one DVE subtract + one 128KB err DMA per group.
"""

import numpy as np
import ml_dtypes
from contextlib import ExitStack

import concourse.bass as bass
import concourse.tile as tile
from concourse import mybir
from concourse.bass_utils import run_bass_kernel_spmd

# ---------------- problem constants (hardcoded per spec) ----------------
S, B, D, H = 512, 256, 256, 1024
NCORES = 8
BS = B // NCORES                  # 32 batch per core
TAU, GAMMA = 10.0, 0.1
INV_TAU = 1.0 / TAU
DT_SCALE = -GAMMA * INV_TAU       # -0.01, the per-step bank delta scale
KT = H // 128                     # 8 contraction tiles
MT = H // 128                     # 8 output tiles
CH = ((0, 3), (3, 3), (6, 2))     # (first m-tile, n m-tiles) per chunk
G = len(CH)
CWs = [nm * BS for _, nm in CH]
NSLOT = 4                         # xp accumulation slots per DMA group
NRING = 12                        # thb ring depth (proj reads <=7 back)
DLAG = 2                          # recompute d = 1-th^2 every DLAG steps

TRACE = False
LAST_RESULTS = None
_BUILT = None


def _split_multi_waits(nc, max_waits: int = 1):
    """Split multi-wait instructions into single-wait NOP chains (the walrus
    build supports one sync-wait slot on CTRL-encoded instructions)."""
    for fn in nc.m.functions:
        for bb in fn.blocks:
            new_insts = []
            for inst in bb.instructions:
                si = inst.sync_info
                if si is not None and len(si.on_wait) > max_waits:
                    waits = list(si.on_wait)
                    for w in waits[:-max_waits]:
                        nop = mybir.InstNoOp(
                            name=nc.get_next_instruction_name(), ins=[], outs=[])
                        nop.engine = inst.engine
                        nop.sync_info = mybir.SyncInfo(on_wait=[w], on_update=[])
                        nc.register_instruction(nop)
                        new_insts.append(nop)
                    si.on_wait = waits[-max_waits:]
                new_insts.append(inst)
            bb.instructions = new_insts


def _chunk_of_k(k):
    for c, (m0, nm) in enumerate(CH):
        if m0 <= k < m0 + nm:
            return c, k - m0
    raise AssertionError


def _build_bass():
    nc = bass.Bass("TRN2", target_bir_lowering=False, debug=False,
                   num_devices=NCORES)
    dt = mybir.dt
    f32, bf16 = dt.float32, dt.bfloat16

    A_d = nc.dram_tensor("A", [128, KT * MT * 128], bf16, kind="ExternalInput").ap()
    Wo_d = nc.dram_tensor("Wo", [128, KT * D], bf16, kind="ExternalInput").ap()
    Dg_d = nc.dram_tensor("Dg", [128, 128], bf16, kind="ExternalInput").ap()
    th0_d = nc.dram_tensor("th0", [128, MT * BS], bf16, kind="ExternalInput").ap()
    thm0_d = nc.dram_tensor("thm0", [128, MT * BS], f32, kind="ExternalInput").ap()
    D0_d = nc.dram_tensor("D0", [128, MT * BS], bf16, kind="ExternalInput").ap()
    T0_d = nc.dram_tensor("T0", [128, MT * BS], f32, kind="ExternalInput").ap()
    x_d = nc.dram_tensor("x", [S, BS, D], f32, kind="ExternalInput").ap()
    err_d = nc.dram_tensor("err", [S, BS, D], f32, kind="ExternalOutput").ap()

    Tanh = mybir.ActivationFunctionType.Tanh
    Square = mybir.ActivationFunctionType.Square
    MUL, ADD, SUB = (mybir.AluOpType.mult, mybir.AluOpType.add,
                     mybir.AluOpType.subtract)

    x_g = x_d.rearrange("(g s) b d -> g (s b) d", s=NSLOT)
    e_g = err_d.rearrange("(g s) b d -> g (s b) d", s=NSLOT)

    with tile.TileContext(nc) as tc, ExitStack() as ctx:
        const = ctx.enter_context(tc.tile_pool(name="const", bufs=1))
        state = ctx.enter_context(tc.tile_pool(name="state", bufs=1))
        upool = ctx.enter_context(tc.tile_pool(name="us", bufs=2))
        mpool = ctx.enter_context(tc.tile_pool(name="ms", bufs=2))
        sqpool = ctx.enter_context(tc.tile_pool(name="sqs", bufs=1))
        zpool = ctx.enter_context(tc.tile_pool(name="zps", bufs=1, space="PSUM"))
        xppool = ctx.enter_context(tc.tile_pool(name="xpps", bufs=2, space="PSUM"))
        xtp = ctx.enter_context(tc.tile_pool(name="xt", bufs=4))
        etp = ctx.enter_context(tc.tile_pool(name="et", bufs=3))

        A_sb = const.tile([128, KT * MT * 128], bf16, tag="A", name="A_sb")
        Wo_sb = const.tile([128, KT * D], bf16, tag="Wo", name="Wo_sb")
        Dg_sb = const.tile([128, 128], bf16, tag="Dg", name="Dg_sb")
        T0_sb = const.tile([128, MT * BS], f32, tag="T0", name="T0_sb")
        ones = const.tile([128, max(CWs)], bf16, tag="ones", name="ones")
        zeros = const.tile([128, 128], bf16, tag="zeros", name="zeros")
        nc.sync.dma_start(A_sb[:], A_d[:])
        nc.sync.dma_start(Wo_sb[:], Wo_d[:])
        nc.sync.dma_start(Dg_sb[:], Dg_d[:])
        nc.sync.dma_start(T0_sb[:], T0_d[:])
        nc.vector.memset(ones[:], INV_TAU)
        nc.vector.memset(zeros[:], 0.0)

        thm = [state.tile([128, CWs[c]], f32, tag=f"thm{c}", name=f"thm{c}")
               for c in range(G)]
        dts = [state.tile([128, CWs[c]], bf16, tag=f"d{c}", name=f"d{c}")
               for c in range(G)]
        TH = [[state.tile([128, CWs[c]], bf16, tag=f"TH{r}_{c}", name=f"TH{r}_{c}")
               for c in range(G)] for r in range(NRING)]
        zT = [zpool.tile([128, CWs[c]], f32, tag=f"z{c}", name=f"zT{c}")
              for c in range(G)]

        offs = []
        off = 0
        for c in range(G):
            offs.append(off)
            sl = slice(off, off + CWs[c])
            nc.sync.dma_start(thm[c][:], thm0_d[:, sl])
            nc.sync.dma_start(dts[c][:], D0_d[:, sl])
            nc.sync.dma_start(TH[NRING - 1][c][:], th0_d[:, sl])
            off += CWs[c]

        # m_{-1} := th0 (step-0 stream computes z_0 = t_0 + A^T th_{-1})
        m_prev = []
        for c in range(G):
            mt = mpool.tile([128, CWs[c]], bf16, tag=f"m{c}", name=f"m{c}")
            nc.sync.dma_start(mt[:], th0_d[:, offs[c]:offs[c] + CWs[c]])
            m_prev.append(mt)
        u_prev = [None] * G

        # Prime PSUM has_written bits once, then write t_0; all step matmuls
        # accumulate on top forever (start=False, stop=False).
        for c in range(G):
            nc.tensor.matmul(zT[c][:], lhsT=zeros[:], rhs=zeros[:, :CWs[c]],
                             start=True, stop=True)
        for c in range(G):
            nc.vector.scalar_tensor_tensor(
                zT[c][:], T0_sb[:, offs[c]:offs[c] + CWs[c]], 0.0,
                T0_sb[:, offs[c]:offs[c] + CWs[c]], MUL, ADD)

        def m_slice(tiles, k):
            c, o = _chunk_of_k(k)
            return tiles[c][:, o * BS:(o + 1) * BS]

        def thb_slice(r, k):
            c, o = _chunk_of_k(k)
            return TH[r][c][:, o * BS:(o + 1) * BS]

        BLOCKS = ((0, 0), (1, 0), (0, 1), (0, 2), (1, 1), (1, 2),
                  (2, 0), (2, 1), (2, 2))
        # block index after which each row completes (diag MMs emitted there)
        ROW_END = {0: 3, 1: 5, 2: 8}

        def emit_rec(j):
            for bi, (c, dch) in enumerate(BLOCKS):
                m0, nm = CH[c]
                k0, nk = CH[dch]
                z = zT[c]
                for m in range(m0, m0 + nm):
                    mo = m - m0
                    for k in range(k0, k0 + nk):
                        nc.tensor.matmul(
                            z[:, mo * BS:(mo + 1) * BS],
                            lhsT=A_sb[:, (k * MT + m) * 128:(k * MT + m + 1) * 128],
                            rhs=m_slice(m_prev, k),
                            start=False, stop=False,
                            skip_group_check=True)
                if ROW_END[c] == bi and j > 0:
                    # bank delta: z_c += -(G/TAU) * u_{j-1} (one MM per chunk
                    # -- the diagonal acts on the partition dim, so the full
                    # chunk width streams through in a single matmul)
                    nc.tensor.matmul(
                        z[:], lhsT=Dg_sb[:], rhs=u_prev[c][:],
                        start=False, stop=False, skip_group_check=True)

        def emit_chain(j):
            nonlocal m_prev, u_prev
            wr = j % NRING
            us, ms = [], []
            for c in range(G):
                u = upool.tile([128, CWs[c]], bf16, tag=f"u{c}", name=f"u{c}")
                nc.scalar.activation(u[:], zT[c][:], Tanh)
                m = mpool.tile([128, CWs[c]], bf16, tag=f"m{c}", name=f"m{c}")
                nc.vector.tensor_tensor(m[:], u[:], dts[c][:], MUL)
                us.append(u)
                ms.append(m)
            for c in range(G):   # bf16 ring copy: thb = thm_old + m (DVE)
                nc.vector.tensor_tensor(TH[wr][c][:], thm[c][:], ms[c][:], ADD)
            for c in range(G):   # fp32 master: thm += m (GPSIMD)
                nc.gpsimd.tensor_tensor(thm[c][:], thm[c][:], ms[c][:], ADD)
            if j < S - 1:
                # refresh d_c every DLAG steps, staggered across parities so
                # the ACT/DVE load is flat per step (1.5 sq+d per step)
                for c in range(G):
                    if (j + c) % DLAG != 0:
                        continue
                    sq = sqpool.tile([128, CWs[c]], bf16, tag=f"sq{c}",
                                     name=f"sq{c}")
                    # read the bf16 ring copy (ready right after the DVE thb
                    # add) rather than thm (ready only after the late GPSIMD
                    # master add) -- a thm read here head-of-line-blocks the
                    # ACT queue and delays the next step's ring u by ~400ns
                    nc.scalar.activation(sq[:], TH[wr][c][:], Square)
                    nc.vector.scalar_tensor_tensor(
                        dts[c][:], sq[:], -INV_TAU, ones[:, :CWs[c]], MUL, ADD)
            m_prev, u_prev = ms, us

        def emit_proj_burst(g, k, xp):
            for s in range(NSLOT):
                nc.tensor.matmul(
                    xp[32 * s:32 * (s + 1), :],
                    lhsT=thb_slice((4 * g + s) % NRING, k),
                    rhs=Wo_sb[:, k * D:(k + 1) * D],
                    start=(k == 0), stop=(k == KT - 1),
                    tile_position=(0, 32 * s))

        xt_tiles = {}

        def emit_group_out(g, xp):
            et = etp.tile([128, D], f32, tag="et", name="et")
            nc.vector.scalar_tensor_tensor(
                et[:], xp[:], 0.0, xt_tiles.pop(g)[:], ADD, SUB)
            nc.sync.dma_start(e_g[g], et[:])

        def prefetch_x(g):
            xt = xtp.tile([128, D], f32, tag="xt", name="xt")
            nc.sync.dma_start(xt[:], x_g[g])
            xt_tiles[g] = xt

        for g0 in range(4):
            prefetch_x(g0)
        xp_tiles = {}
        for j in range(S):
            emit_rec(j)
            if j >= 4:
                gp = (j - 4) // NSLOT
                poff = ((j - 4) % NSLOT) * 2
                if poff == 0:
                    xp_tiles[gp] = xppool.tile([128, D], f32, tag="xp",
                                               name="xp")
                for k in (poff, poff + 1):
                    emit_proj_burst(gp, k, xp_tiles[gp])
            emit_chain(j)
            if j % NSLOT == 0 and j >= 8:
                g_done = (j - 8) // NSLOT
                emit_group_out(g_done, xp_tiles.pop(g_done))
                pf = j // NSLOT + 2
                if pf <= S // NSLOT - 1:
                    prefetch_x(pf)
        g = S // NSLOT - 1
        xp_tiles[g] = xppool.tile([128, D], f32, tag="xp", name="xp")
        for k in range(KT):
            emit_proj_burst(g, k, xp_tiles[g])
        emit_group_out(S // NSLOT - 2, xp_tiles.pop(S // NSLOT - 2))
        emit_group_out(g, xp_tiles.pop(g))

    _split_multi_waits(nc)
    return nc


def _host_prep(x, h_init, v_r, b_r, w_o, b_o):
    """Build per-core input maps (all layout work in numpy)."""
    x = np.asarray(x, np.float32)
    h_init = np.asarray(h_init, np.float32)
    v_r = np.asarray(v_r, np.float32)
    b_r = np.asarray(b_r, np.float32)
    w_o = np.asarray(w_o, np.float32)
    b_o = np.asarray(b_o, np.float32)

    mask = np.tril(np.ones((H, H), np.float32), -1)
    w_r = v_r * mask
    A = w_r - w_r.T                                           # [H, H]
    A_sb = np.ascontiguousarray(
        A.reshape(KT, 128, MT, 128).transpose(1, 0, 2, 3).reshape(128, KT * MT * 128)
    ).astype(ml_dtypes.bfloat16)
    Wo_sb = np.ascontiguousarray(
        w_o.T.reshape(KT, 128, D).transpose(1, 0, 2).reshape(128, KT * D)
    ).astype(ml_dtypes.bfloat16)
    Dg = (DT_SCALE * np.eye(128, dtype=np.float32)).astype(ml_dtypes.bfloat16)

    in_maps = []
    for c in range(NCORES):
        hc = h_init[c * BS:(c + 1) * BS]                       # [BS, H]
        h0 = np.ascontiguousarray(
            hc.reshape(BS, MT, 128).transpose(2, 1, 0)         # [128, MT, BS]
        ).reshape(128, MT * BS).astype(np.float32)
        thm0 = np.tanh(h0).astype(np.float32)
        th0 = thm0.astype(ml_dtypes.bfloat16)
        sq0 = (thm0 * thm0).astype(ml_dtypes.bfloat16).astype(np.float32)
        D0 = (INV_TAU * (1.0 - sq0)).astype(ml_dtypes.bfloat16)
        t0 = (np.broadcast_to(
            b_r.reshape(MT, 128, 1).transpose(1, 0, 2), (128, MT, BS))
            .reshape(128, MT * BS) - GAMMA * h0).astype(np.float32)
        in_maps.append({
            "A": A_sb, "Wo": Wo_sb, "Dg": Dg,
            "th0": th0, "thm0": thm0, "D0": D0,
            "T0": np.ascontiguousarray(t0),
            "x": np.ascontiguousarray(x[:, c * BS:(c + 1) * BS, :] - b_o),
        })
    return in_maps


def kernel(x, h_init, v_r, b_r, w_o, b_o):
    global _BUILT, LAST_RESULTS
    if _BUILT is None:
        _BUILT = _build_bass()
    nc = _BUILT
    in_maps = _host_prep(x, h_init, v_r, b_r, w_o, b_o)
    res = run_bass_kernel_spmd(nc, in_maps, core_ids=list(range(NCORES)),
                               trace=TRACE)
    LAST_RESULTS = res
    out = np.empty((S, B, D), np.float32)
    for c in range(NCORES):
        out[:, c * BS:(c + 1) * BS, :] = np.asarray(res.results[c]["err"])
    return out
